# revision 7
# baseline (speedup 1.0000x reference)
"""CLIP causal attention (B=8, T=1024, E=768, H=12) on 8 TRN2 NeuronCores.

Strategy: pure data-parallel over batch — core b handles x[b] end to end,
no collectives. All compute in transposed space (embed on partitions):

  X' = x_b^T                       [768, 1024]  (host pre-transposed, bf16)
  Q' = Wq^T @ X' (+bq)             [768, 1024]  lhsT = Wq as stored
  K' = Wk^T @ X' (+bk)             [768, 1024]
  V  = X'^T @ Wv (+bv)             [1024, 768]  lhsT = X' blocks (j on partitions)
  per head h (KQ orientation, j on partitions, i free):
     S'[j,i] = K'_h[:,jblk]^T @ Q'_h          (K=64)
     P' = exp(S' * 1/8)  (no max-subtraction: |S'/8| <= ~7, exact-safe)
     causal: skip fully-masked blocks, restrict to valid cols, tri-mask diag
     O_aug[d,i] = sum_j Vaug_h[j,d]^T @ P'    (Vaug has a ones column ->
                                               row 64 = softmax denominator)
     O'_h = O_aug[0:64] * broadcast(1/denom)
  out = (O'^T @ Wo) + bo           [1024, 768]  lhsT = O' blocks -> direct
                                                untransposed output

Head-PAIR scheduling: the two heads of each 128-row Q/K block live at
partition offsets 0 and 64, so their K=64 score matmuls auto-derive PE
tile_position (0,0)/(64,0). Emitting them back-to-back lets the PE run
both CONCURRENTLY on disjoint row-groups (~2x on scores). The PV matmuls
of j-tile jt are emitted after the scores of jt+1 (software pipeline), so
the exp()/tri-mask latency of tile jt hides under the scores of jt+1.
PSUM: one pool, tag "s" = 2x[128,1024] slots (scores pairs + all
projection accumulators share them), tag "o" = 4x[128,512] (the pair's
four O_aug accumulators; reused by the out-projection). The final
out-projection is emitted right after the last pair: its first i-blocks
depend only on the o_ps0 normalizes (done at jt==3), so it fills the PE
while the last pair's o_ps1 normalize chain drains — no filler matmuls
needed in the tail. All matmul operands bf16 (fp32 PSUM accumulation);
measured end-to-end rel l2 err vs fp32 reference ~5e-3.
"""

import numpy as np
import ml_dtypes

E = 768
T = 1024
B = 8
H = 12
DH = 64
NT = E // 128          # 6 partition-tiles of the embed dim
NJ = T // 128          # 8 partition-tiles of the token dim
SCALE = DH ** -0.5     # folded into the exp() activation's scale operand
VW = H * 128           # V_aug row width: 12 heads x 128 cols (64 data +
                       # ones col + zero pad so the PV stationary operand
                       # is a full 128x128 block -> fast weight load)

_CACHE = {}


def _build():
    import concourse.bass as bass
    import concourse.tile as tile
    from concourse import bacc, mybir

    f32 = mybir.dt.float32
    bf16 = mybir.dt.bfloat16
    Exp = mybir.ActivationFunctionType.Exp

    nc = bacc.Bacc(
        "TRN2",
        target_bir_lowering=False,
        debug=False,
        enable_asserts=False,
        num_devices=B,
    )

    xt = nc.dram_tensor("xt", [E, T], bf16, kind="ExternalInput").ap()
    wq = nc.dram_tensor("wq", [E, E], bf16, kind="ExternalInput").ap()
    wk = nc.dram_tensor("wk", [E, E], bf16, kind="ExternalInput").ap()
    wv = nc.dram_tensor("wv", [E, E], bf16, kind="ExternalInput").ap()
    wo = nc.dram_tensor("wo", [E, E], bf16, kind="ExternalInput").ap()
    bqt = nc.dram_tensor("bqt", [128, NT], f32, kind="ExternalInput").ap()
    bkt = nc.dram_tensor("bkt", [128, NT], f32, kind="ExternalInput").ap()
    bvr = nc.dram_tensor("bvr", [1, E], bf16, kind="ExternalInput").ap()
    bor = nc.dram_tensor("bor", [1, E], bf16, kind="ExternalInput").ap()
    tri = nc.dram_tensor("tri", [128, 128], bf16, kind="ExternalInput").ap()
    ones12 = nc.dram_tensor("ones12", [128, NJ * H], bf16, kind="ExternalInput").ap()
    out = nc.dram_tensor("out", [T, E], f32, kind="ExternalOutput").ap()

    with tile.TileContext(nc) as tc:
        with (
            tc.tile_pool(name="const", bufs=1) as cpool,
            tc.tile_pool(name="psb", bufs=6) as ppool,
            tc.tile_pool(name="rsb", bufs=4) as rpool,
            tc.tile_pool(name="rbsb", bufs=4) as rbpool,
            tc.tile_pool(name="fin", bufs=3) as fpool,
            tc.tile_pool(name="ps", bufs=4, space="PSUM") as psp,
        ):
            XT = cpool.tile([128, NT * T], bf16)     # (kt, i)
            WQ = cpool.tile([128, NT * E], bf16)     # (kt, n)
            WK = cpool.tile([128, NT * E], bf16)
            WV = cpool.tile([128, NT * E], bf16)
            WO = cpool.tile([128, NT * E], bf16)
            QS = cpool.tile([128, NT * T], bf16)     # Q' (nt, i)
            KS = cpool.tile([128, NT * T], bf16)
            VS = cpool.tile([128, NJ * VW], bf16)    # (jt, h*128+d); col 64 of
                                                     # each head block = ones
            OS = cpool.tile([128, NT * T], bf16)     # O' (et, i)
            BQ = cpool.tile([128, NT], f32)
            BK = cpool.tile([128, NT], f32)
            BVR = cpool.tile([1, E], bf16)
            BOR = cpool.tile([1, E], bf16)
            TRI = cpool.tile([128, 128], bf16)

            # PSUM: tag "s" = 2 slots of [128,1024] f32 (4 banks) shared by
            # the paired score tiles and every projection accumulator; tag
            # "o" = 4 slots of [128,512] (4 banks) for the pair's O_aug
            # accumulators and the out-projection groups.
            def ps_s(cols=512):
                s = psp.tile([128, cols], f32, tag="s", bufs=2,
                             padded_shape=[128, 1024], name="sps")
                return s

            def ps_o():
                o = psp.tile([128, 512], f32, tag="o", bufs=4, name="ops")
                return o

            # ---- input DMAs: tiny constants first (Q/K/V evictions need the
            # biases; don't queue them behind 4.5MB of weights), then
            # per-k-tile splits so compute can start early, spread across the
            # three DMA-capable issue queues (sync/scalar/gpsimd). WV before
            # WQ/WK: the V projection (XT + WV only) runs first. ----
            nc.sync.dma_start(BQ[:], bqt)
            nc.sync.dma_start(BK[:], bkt)
            nc.sync.dma_start(BVR[:], bvr)
            nc.sync.dma_start(BOR[:], bor)
            nc.sync.dma_start(TRI[:], tri)
            BVB = cpool.tile([128, E], bf16)
            FINB = cpool.tile([128, E], bf16)
            nc.gpsimd.partition_broadcast(BVB[:], BVR[:])
            nc.gpsimd.partition_broadcast(FINB[:], BOR[:])
            xt3 = xt.rearrange("(k p) i -> p k i", p=128)
            w3 = {
                id(WQ): wq.rearrange("(k p) n -> p k n", p=128),
                id(WK): wk.rearrange("(k p) n -> p k n", p=128),
                id(WV): wv.rearrange("(k p) n -> p k n", p=128),
                id(WO): wo.rearrange("(k p) n -> p k n", p=128),
            }
            for kt in range(NT):
                nc.sync.dma_start(XT[:, kt * T : (kt + 1) * T], xt3[:, kt])
                nc.scalar.dma_start(WV[:, kt * E : (kt + 1) * E], w3[id(WV)][:, kt])
            dmaq = [nc.sync, nc.scalar, nc.gpsimd]
            qi = 0
            for W in (WQ, WK, WO):
                for kt in range(NT):
                    dmaq[qi % 3].dma_start(
                        W[:, kt * E : (kt + 1) * E], w3[id(W)][:, kt]
                    )
                    qi += 1
            # ---- PE warmup: dummy matmuls with no DMA dependency so the
            # HAM activity monitor lifts the 1.2GHz cold gate before real
            # work arrives (DUM memset first: it gates the dummies) ----
            DUMW = cpool.tile([128, 128], bf16)
            DUMR = cpool.tile([128, 512], bf16)
            nc.vector.memset(DUMW[:], 1.0)
            nc.vector.memset(DUMR[:], 1.0)

            def dummy(n=512):
                # full-array junk matmul: the HAM activity monitor only lifts
                # the 1.2GHz cold gate for real array occupancy.
                d_ps = ps_s()
                nc.tensor.matmul(
                    d_ps[:, :n], lhsT=DUMW[:], rhs=DUMR[:, :n], start=True, stop=True
                )

            for _ in range(16):
                dummy()
            # V_aug ones column via one strided DMA (strided DVE memsets
            # cost ~1us each and would delay the V-projection evictions).
            # Pad columns 65..127 only feed PSUM rows that are never read,
            # but zero them anyway (one cheap contiguous memset) so the
            # simulator's uninitialized-read check stays green.
            nc.vector.memset(VS[:], 0.0)
            nc.sync.dma_start(
                VS[:].rearrange("p (j h e) -> p j h e", h=H, e=128)[:, :, :, 64:65],
                ones12.rearrange("p (j h e) -> p j h e", h=H, e=1),
            )

            # ---- V projection: lhsT = X'[kt, jblk] -> V[j, e] + ones-col layout.
            # Groups rotate over all six PSUM slots (2x"s" + 4x"o") so the
            # ~700ns DVE evictions never gate the accumulation pipeline.
            vslot = 0
            for jt in range(NJ):
                for e0, ew, h0, nh in ((0, 512, 0, 8), (512, 256, 8, 4)):
                    ps = ps_s() if vslot % 6 < 2 else ps_o()
                    vslot += 1
                    for kt in range(NT):
                        nc.tensor.matmul(
                            ps[:, :ew],
                            lhsT=XT[:, kt * T + jt * 128 : kt * T + jt * 128 + 128],
                            rhs=WV[:, kt * E + e0 : kt * E + e0 + ew],
                            start=(kt == 0),
                            stop=(kt == NT - 1),
                        )
                    dst = (
                        VS[:, jt * VW + h0 * 128 : jt * VW + (h0 + nh) * 128]
                        .rearrange("p (h e) -> p h e", e=128)[:, :, 0:64]
                    )
                    nc.vector.tensor_add(
                        dst,
                        ps[:, :ew].rearrange("p (h d) -> p h d", d=64),
                        BVB[:, e0 : e0 + ew].rearrange("p (h d) -> p h d", d=64),
                    )

            # ---- Q'/K' projection for one 128-row block nt (2 heads) ----
            def qk_proj(nt):
                for W, Bb, DST in ((WQ, BQ, QS), (WK, BK, KS)):
                    for ic in range(2):
                        ps = ps_s()
                        for kt in range(NT):
                            nc.tensor.matmul(
                                ps[:],
                                lhsT=W[:, kt * E + nt * 128 : kt * E + nt * 128 + 128],
                                rhs=XT[:, kt * T + ic * 512 : kt * T + ic * 512 + 512],
                                start=(kt == 0),
                                stop=(kt == NT - 1),
                            )
                        # eviction on ScalarE (ACT Identity + per-partition
                        # bias AP): keeps the DVE FIFO free so PSUM slot
                        # recycling never stalls the PE's next group
                        nc.scalar.add(
                            DST[:, nt * T + ic * 512 : nt * T + ic * 512 + 512],
                            ps[:],
                            Bb[:, nt : nt + 1],
                        )

            def norm_prep(o_ps):
                # softmax denominators live in row 64 (the V_aug ones column).
                # Full-precision reciprocal costs 3.35us on DVE; the ~18-bit
                # approx is plenty, but its BITWISE_NOT seed needs an SBUF
                # operand on hardware, so stage the PSUM row out first.
                dn = rpool.tile([1, 512], f32, tag="denom")
                nc.vector.tensor_copy(dn[:], o_ps[64:65, :])
                r = rpool.tile([1, 512], f32, tag="recip")
                nc.vector.reciprocal_approx_fast(r[:], dn[:])
                rb = rbpool.tile([64, 512], f32, tag="recipb")
                nc.gpsimd.partition_broadcast(rb[:], r[:])
                return rb

            def norm_mul(o_ps, rb, h, ic):
                # deferred so the DVE FIFO never sits blocked on the ~1us
                # gpsimd broadcast while tri-masks/evictions queue behind it
                nt, po = h // 2, (h % 2) * 64
                nc.vector.tensor_mul(
                    OS[po : po + 64, nt * T + ic * 512 : nt * T + ic * 512 + 512],
                    o_ps[0:64, :],
                    rb[:],
                )

            # ---- attention for a head PAIR (2nt, 2nt+1), KQ orientation.
            # Per j-tile, head A's and head B's K=64 score matmuls are
            # emitted interleaved so the PE runs them concurrently on row
            # groups (0,0)/(64,0). PV of tile jt is flushed after the scores
            # of jt+1 so the exp/tri chain hides under PE work. o_ps0
            # (i<512) finishes at jt=3; its normalize overlaps the tail. ----
            def head_pair(nt):
                hA, hB = 2 * nt, 2 * nt + 1
                oA0, oA1 = ps_o(), ps_o()
                oB0, oB1 = ps_o(), ps_o()
                opair = ((hA, oA0, oA1), (hB, oB0, oB1))
                rbs = {}

                def flush_pv(jt, probs):
                    d0 = jt * 128
                    for (h, o0, o1), p in zip(opair, probs):
                        lhsV = VS[:, jt * VW + h * 128 : jt * VW + h * 128 + 128]
                        if jt < 4:
                            nc.tensor.matmul(
                                o0[:, d0:512],
                                lhsT=lhsV,
                                rhs=p[:, d0:512],
                                start=(jt == 0),
                                stop=(jt == 3),
                                skip_group_check=True,
                            )
                        nc.tensor.matmul(
                            o1[:, max(0, d0 - 512) : 512],
                            lhsT=lhsV,
                            rhs=p[:, max(512, d0) : 1024],
                            start=(jt == 0),
                            stop=(jt == NJ - 1),
                            skip_group_check=True,
                        )
                    if jt == 3:
                        rbs["A0"] = norm_prep(oA0)
                        rbs["B0"] = norm_prep(oB0)

                pending = None
                for jt in range(NJ):
                    d0 = jt * 128
                    sA = ps_s(1024)
                    sB = ps_s(1024)
                    pA = ppool.tile([128, 1024], bf16, tag="probs", name="pA")
                    pB = ppool.tile([128, 1024], bf16, tag="probs", name="pB")
                    spair = ((0, sA, pA), (64, sB, pB))
                    if jt < 4:
                        for c0, c1 in ((d0, 512), (512, 1024)):
                            for po, s, _ in spair:
                                nc.tensor.matmul(
                                    s[:, c0:c1],
                                    lhsT=KS[po : po + 64,
                                            nt * T + d0 : nt * T + d0 + 128],
                                    rhs=QS[po : po + 64, nt * T + c0 : nt * T + c1],
                                    start=True,
                                    stop=True,
                                )
                    else:
                        for po, s, _ in spair:
                            nc.tensor.matmul(
                                s[:, d0:1024],
                                lhsT=KS[po : po + 64,
                                        nt * T + d0 : nt * T + d0 + 128],
                                rhs=QS[po : po + 64, nt * T + d0 : nt * T + 1024],
                                start=True,
                                stop=True,
                            )
                    # exp, chunked at the PSUM bank boundary: the jt+1 score
                    # matmul's WAR on this slot releases per-bank (subtile
                    # deps), so the next tile's first chunk starts sooner
                    for po, s, p in spair:
                        if d0 < 512:
                            nc.scalar.activation(p[:, d0:512], s[:, d0:512], Exp, scale=SCALE)
                        nc.scalar.activation(
                            p[:, max(512, d0) : 1024], s[:, max(512, d0) : 1024],
                            Exp, scale=SCALE,
                        )
                    # causal triangle on the diag block. NOT gpsimd
                    # affine_select: mixing custom-op types on GpSimd forces
                    # MODIFY_POOL_CONFIG switches that stall partition_broadcast
                    nc.vector.tensor_mul(pA[:, d0 : d0 + 128], pA[:, d0 : d0 + 128], TRI[:])
                    nc.vector.tensor_mul(pB[:, d0 : d0 + 128], pB[:, d0 : d0 + 128], TRI[:])
                    if pending is not None:
                        flush_pv(*pending)
                    pending = (jt, (pA, pB))
                flush_pv(*pending)
                # ic0 muls first (their broadcasts completed back at jt==3):
                # the out-projection's i-blocks 0..3 need only these, so the
                # tail unblocks ~2us sooner after the last pair
                norm_mul(oA0, rbs["A0"], hA, 0)
                norm_mul(oB0, rbs["B0"], hB, 0)
                rbs["A1"] = norm_prep(oA1)
                rbs["B1"] = norm_prep(oB1)
                norm_mul(oA1, rbs["A1"], hA, 1)
                norm_mul(oB1, rbs["B1"], hB, 1)

            for nt in range(NT):
                qk_proj(nt)
                head_pair(nt)

            # ---- output projection: lhsT = O'[et, iblk] -> out[i, n] directly.
            # i-blocks 0..3 need only the o_ps0 normalizes (done at jt==3 of
            # each pair), so they fill the PE while the last pair's o_ps1
            # normalize chain completes; i-blocks 4..7 follow.
            for it in range(NJ):
                fin = fpool.tile([128, E], f32, tag="fin")
                for n0, nw in ((0, 512), (512, 256)):
                    f_ps = ps_o()
                    for et in range(NT):
                        nc.tensor.matmul(
                            f_ps[:, :nw],
                            lhsT=OS[:, et * T + it * 128 : et * T + it * 128 + 128],
                            rhs=WO[:, et * E + n0 : et * E + n0 + nw],
                            start=(et == 0),
                            stop=(et == NT - 1),
                        )
                    nc.vector.tensor_add(
                        fin[:, n0 : n0 + nw], f_ps[:, :nw], FINB[:, n0 : n0 + nw]
                    )
                    (nc.sync if it % 2 == 0 else nc.scalar).dma_start(
                        out[it * 128 : (it + 1) * 128, n0 : n0 + nw],
                        fin[:, n0 : n0 + nw],
                    )

    nc.compile()
    return nc


def _get_nc():
    if "nc" not in _CACHE:
        _CACHE["nc"] = _build()
    return _CACHE["nc"]


def _make_in_maps(inputs):
    bf = ml_dtypes.bfloat16
    x = np.asarray(inputs["x"], np.float32)
    shared = {
        "wq": np.asarray(inputs["Wq"], np.float32).astype(bf),
        "wk": np.asarray(inputs["Wk"], np.float32).astype(bf),
        "wv": np.asarray(inputs["Wv"], np.float32).astype(bf),
        "wo": np.asarray(inputs["Wo"], np.float32).astype(bf),
        "bqt": np.ascontiguousarray(
            np.asarray(inputs["bq"], np.float32).reshape(NT, 128).T
        ),
        "bkt": np.ascontiguousarray(
            np.asarray(inputs["bk"], np.float32).reshape(NT, 128).T
        ),
        "bvr": np.asarray(inputs["bv"], np.float32).reshape(1, E).astype(bf),
        "bor": np.asarray(inputs["bo"], np.float32).reshape(1, E).astype(bf),
        "tri": np.triu(np.ones((128, 128), np.float32)).astype(bf),
        "ones12": np.ones((128, NJ * H), np.float32).astype(bf),
    }
    return [dict(shared, xt=x[b].T.astype(bf)) for b in range(B)]


def _run(inputs, trace=False):
    from concourse import bass_utils

    nc = _get_nc()
    res = bass_utils.run_bass_kernel_spmd(
        nc, _make_in_maps(inputs), core_ids=list(range(B)), trace=trace
    )
    out = np.stack([np.asarray(res.results[c]["out"]) for c in range(B)])
    return out, res


def kernel(**inputs) -> np.ndarray:
    out, _ = _run(inputs, trace=False)
    return out


# revision 8
# speedup vs baseline: 1.1046x; 1.1046x over previous
"""CLIP causal attention (B=8, T=1024, E=768, H=12) on 8 TRN2 NeuronCores.

Strategy: pure data-parallel over batch — core b handles x[b] end to end,
no collectives. All compute in transposed space (embed on partitions):

  X' = x_b^T                       [768, 1024]  (host pre-transposed, bf16)
  Q' = Wq^T @ X' (+bq)             [768, 1024]  lhsT = Wq as stored
  K' = Wk^T @ X' (+bk)             [768, 1024]
  V  = X'^T @ Wv (+bv)             [1024, 768]  lhsT = X' blocks (j on partitions)
  per head h (KQ orientation, j on partitions, i free):
     S'[j,i] = K'_h[:,jblk]^T @ Q'_h          (K=64)
     P' = exp(S' * 1/8)  (no max-subtraction: |S'/8| <= ~7, exact-safe)
     causal: skip fully-masked blocks, restrict to valid cols, tri-mask diag
     O_aug[d,i] = sum_j Vaug_h[j,d]^T @ P'    (Vaug has a ones column ->
                                               row 64 = softmax denominator)
     O'_h = O_aug[0:64] * broadcast(1/denom)
  out = (O'^T @ Wo) + bo           [1024, 768]  lhsT = O' blocks -> direct
                                                untransposed output

Head-PAIR scheduling: the two heads of each 128-row Q/K block live at
partition offsets 0 and 64, so their K=64 score matmuls auto-derive PE
tile_position (0,0)/(64,0); emitted back-to-back they run CONCURRENTLY
on disjoint PE row-groups (~2x on scores). Each head's i-range is
processed in two passes (pass0: i<512 over j-tiles 0..3 into O_aug0;
pass1: i>=512 over j-tiles 0..7 into O_aug1) so every score tile is one
PSUM bank wide. PSUM = 4x[128,512] score slots (2-deep pipelining for
the pair; also rotated 4-deep by the projection accumulators, the
V/out-projections and warmup) + 4x[128,512] O_aug accumulators (full
pair lifetime, so no write ever waits on the ~1us gpsimd reciprocal
broadcast). PV of tile jt is flushed after the scores of jt+1, hiding
the exp/tri-mask chain; Q/K projection evictions run on ScalarE (ACT
Identity + per-partition bias) keeping the DVE FIFO short. The final
out-projection's first i-blocks depend only on the o_ps0 normalizes
(done mid-pair), so the tail drains without filler matmuls. All matmul
operands bf16 (fp32 PSUM accumulation); measured end-to-end rel l2 err
vs fp32 reference ~5e-3.
"""

import numpy as np
import ml_dtypes

E = 768
T = 1024
B = 8
H = 12
DH = 64
NT = E // 128          # 6 partition-tiles of the embed dim
NJ = T // 128          # 8 partition-tiles of the token dim
SCALE = DH ** -0.5     # folded into the exp() activation's scale operand
VW = H * 128           # V_aug row width: 12 heads x 128 cols (64 data +
                       # ones col + zero pad so the PV stationary operand
                       # is a full 128x128 block -> fast weight load)

_CACHE = {}


def _build():
    import concourse.bass as bass
    import concourse.tile as tile
    from concourse import bacc, mybir

    f32 = mybir.dt.float32
    bf16 = mybir.dt.bfloat16
    Exp = mybir.ActivationFunctionType.Exp

    nc = bacc.Bacc(
        "TRN2",
        target_bir_lowering=False,
        debug=False,
        enable_asserts=False,
        num_devices=B,
    )

    xt = nc.dram_tensor("xt", [E, T], bf16, kind="ExternalInput").ap()
    wq = nc.dram_tensor("wq", [E, E], bf16, kind="ExternalInput").ap()
    wk = nc.dram_tensor("wk", [E, E], bf16, kind="ExternalInput").ap()
    wv = nc.dram_tensor("wv", [E, E], bf16, kind="ExternalInput").ap()
    wo = nc.dram_tensor("wo", [E, E], bf16, kind="ExternalInput").ap()
    bqt = nc.dram_tensor("bqt", [128, NT], f32, kind="ExternalInput").ap()
    bkt = nc.dram_tensor("bkt", [128, NT], f32, kind="ExternalInput").ap()
    bvr = nc.dram_tensor("bvr", [1, E], bf16, kind="ExternalInput").ap()
    bor = nc.dram_tensor("bor", [1, E], bf16, kind="ExternalInput").ap()
    tri = nc.dram_tensor("tri", [128, 128], bf16, kind="ExternalInput").ap()
    ones12 = nc.dram_tensor("ones12", [128, NJ * H], bf16, kind="ExternalInput").ap()
    out = nc.dram_tensor("out", [T, E], f32, kind="ExternalOutput").ap()

    with tile.TileContext(nc) as tc:
        with (
            tc.tile_pool(name="const", bufs=1) as cpool,
            tc.tile_pool(name="psb", bufs=8) as ppool,
            tc.tile_pool(name="rsb", bufs=4) as rpool,
            tc.tile_pool(name="rbsb", bufs=4) as rbpool,
            tc.tile_pool(name="fin", bufs=3) as fpool,
            tc.tile_pool(name="ps", bufs=4, space="PSUM") as psp,
        ):
            XT = cpool.tile([128, NT * T], bf16)     # (kt, i)
            WQ = cpool.tile([128, NT * E], bf16)     # (kt, n)
            WK = cpool.tile([128, NT * E], bf16)
            WV = cpool.tile([128, NT * E], bf16)
            WO = cpool.tile([128, NT * E], bf16)
            QS = cpool.tile([128, NT * T], bf16)     # Q' (nt, i)
            KS = cpool.tile([128, NT * T], bf16)
            VS = cpool.tile([128, NJ * VW], bf16)    # (jt, h*128+d); col 64 of
                                                     # each head block = ones
            OS = cpool.tile([128, NT * T], bf16)     # O' (et, i)
            BQ = cpool.tile([128, NT], f32)
            BK = cpool.tile([128, NT], f32)
            BVR = cpool.tile([1, E], bf16)
            BOR = cpool.tile([1, E], bf16)
            TRI = cpool.tile([128, 128], bf16)

            # PSUM: tag "sc" = 4 x [128,512] (scores / every projection
            # accumulator / warmup), tag "o" = 4 x [128,512] (pair O_augs)
            def ps_sc():
                s = psp.tile([128, 512], f32, tag="sc", bufs=4, name="scps")
                return s

            def ps_o():
                o = psp.tile([128, 512], f32, tag="o", bufs=4, name="ops")
                return o

            # ---- input DMAs: tiny constants first (Q/K/V evictions need the
            # biases; don't queue them behind 4.5MB of weights), then
            # per-k-tile splits so compute can start early, spread across the
            # three DMA-capable issue queues (sync/scalar/gpsimd). WV before
            # WQ/WK: the V projection (XT + WV only) runs first. ----
            nc.sync.dma_start(BQ[:], bqt)
            nc.sync.dma_start(BK[:], bkt)
            nc.sync.dma_start(BVR[:], bvr)
            nc.sync.dma_start(BOR[:], bor)
            nc.sync.dma_start(TRI[:], tri)
            BVB = cpool.tile([128, E], bf16)
            FINB = cpool.tile([128, E], bf16)
            nc.gpsimd.partition_broadcast(BVB[:], BVR[:])
            nc.gpsimd.partition_broadcast(FINB[:], BOR[:])
            xt3 = xt.rearrange("(k p) i -> p k i", p=128)
            w3 = {
                id(WQ): wq.rearrange("(k p) n -> p k n", p=128),
                id(WK): wk.rearrange("(k p) n -> p k n", p=128),
                id(WV): wv.rearrange("(k p) n -> p k n", p=128),
                id(WO): wo.rearrange("(k p) n -> p k n", p=128),
            }
            for kt in range(NT):
                nc.sync.dma_start(XT[:, kt * T : (kt + 1) * T], xt3[:, kt])
                nc.scalar.dma_start(WV[:, kt * E : (kt + 1) * E], w3[id(WV)][:, kt])
            dmaq = [nc.sync, nc.scalar, nc.gpsimd]
            qi = 0
            for W in (WQ, WK, WO):
                for kt in range(NT):
                    dmaq[qi % 3].dma_start(
                        W[:, kt * E : (kt + 1) * E], w3[id(W)][:, kt]
                    )
                    qi += 1
            # ---- PE warmup: dummy matmuls with no DMA dependency so the
            # HAM activity monitor lifts the 1.2GHz cold gate before real
            # work arrives (DUM memset first: it gates the dummies) ----
            DUMW = cpool.tile([128, 128], bf16)
            DUMR = cpool.tile([128, 512], bf16)
            nc.vector.memset(DUMW[:], 1.0)
            nc.vector.memset(DUMR[:], 1.0)

            def dummy(n=512):
                # full-array junk matmul: the HAM activity monitor only lifts
                # the 1.2GHz cold gate for real array occupancy.
                d_ps = ps_sc()
                nc.tensor.matmul(
                    d_ps[:, :n], lhsT=DUMW[:], rhs=DUMR[:, :n], start=True, stop=True
                )

            for _ in range(16):
                dummy()
            # V_aug ones column via one strided DMA (strided DVE memsets
            # cost ~1us each and would delay the V-projection evictions).
            # Pad columns 65..127 only feed PSUM rows that are never read,
            # but zero them anyway (one cheap contiguous memset) so the
            # simulator's uninitialized-read check stays green.
            nc.vector.memset(VS[:], 0.0)
            nc.sync.dma_start(
                VS[:].rearrange("p (j h e) -> p j h e", h=H, e=128)[:, :, :, 64:65],
                ones12.rearrange("p (j h e) -> p j h e", h=H, e=1),
            )

            # ---- V projection: lhsT = X'[kt, jblk] -> V[j, e] + ones-col
            # layout. Groups alternate between the sc and o slot sets
            # (8-deep rotation) so the ~700ns DVE evictions never gate the
            # accumulation pipeline.
            vslot = 0
            for jt in range(NJ):
                for e0, ew, h0, nh in ((0, 512, 0, 8), (512, 256, 8, 4)):
                    ps = ps_sc() if vslot % 2 == 0 else ps_o()
                    vslot += 1
                    for kt in range(NT):
                        nc.tensor.matmul(
                            ps[:, :ew],
                            lhsT=XT[:, kt * T + jt * 128 : kt * T + jt * 128 + 128],
                            rhs=WV[:, kt * E + e0 : kt * E + e0 + ew],
                            start=(kt == 0),
                            stop=(kt == NT - 1),
                        )
                    dst = (
                        VS[:, jt * VW + h0 * 128 : jt * VW + (h0 + nh) * 128]
                        .rearrange("p (h e) -> p h e", e=128)[:, :, 0:64]
                    )
                    nc.vector.tensor_add(
                        dst,
                        ps[:, :ew].rearrange("p (h d) -> p h d", d=64),
                        BVB[:, e0 : e0 + ew].rearrange("p (h d) -> p h d", d=64),
                    )

            # ---- Q'/K' projection for one 128-row block nt (2 heads) ----
            def qk_proj(nt):
                for W, Bb, DST in ((WQ, BQ, QS), (WK, BK, KS)):
                    for ic in range(2):
                        ps = ps_sc()
                        for kt in range(NT):
                            nc.tensor.matmul(
                                ps[:],
                                lhsT=W[:, kt * E + nt * 128 : kt * E + nt * 128 + 128],
                                rhs=XT[:, kt * T + ic * 512 : kt * T + ic * 512 + 512],
                                start=(kt == 0),
                                stop=(kt == NT - 1),
                            )
                        # eviction on ScalarE (ACT Identity + per-partition
                        # bias AP): keeps the DVE FIFO free so PSUM slot
                        # recycling never stalls the PE
                        nc.scalar.add(
                            DST[:, nt * T + ic * 512 : nt * T + ic * 512 + 512],
                            ps[:],
                            Bb[:, nt : nt + 1],
                        )

            def norm_prep(o_ps):
                # softmax denominators live in row 64 (the V_aug ones column).
                # Full-precision reciprocal costs 3.35us on DVE; the ~18-bit
                # approx is plenty, but its BITWISE_NOT seed needs an SBUF
                # operand on hardware, so stage the PSUM row out first.
                dn = rpool.tile([1, 512], f32, tag="denom")
                nc.vector.tensor_copy(dn[:], o_ps[64:65, :])
                r = rpool.tile([1, 512], f32, tag="recip")
                nc.vector.reciprocal_approx_fast(r[:], dn[:])
                rb = rbpool.tile([64, 512], f32, tag="recipb")
                nc.gpsimd.partition_broadcast(rb[:], r[:])
                return rb

            def norm_mul(o_ps, rb, h, ic):
                # separate from norm_prep so the DVE FIFO never sits blocked
                # on the ~1us gpsimd broadcast with work queued behind it
                nt, po = h // 2, (h % 2) * 64
                nc.vector.tensor_mul(
                    OS[po : po + 64, nt * T + ic * 512 : nt * T + ic * 512 + 512],
                    o_ps[0:64, :],
                    rb[:],
                )

            # ---- attention for a head PAIR (2nt, 2nt+1), KQ orientation,
            # i-range in two passes so score tiles are one bank wide:
            #   pass 0: i in [0,512),    j-tiles 0..3, accumulate O_aug0
            #   pass 1: i in [512,1024), j-tiles 0..7, accumulate O_aug1
            # Per j-tile, head A's and head B's K=64 score matmuls are
            # emitted interleaved -> concurrent on PE row groups. PV of
            # tile jt flushes after the scores of the next tile, hiding the
            # exp/tri chain under PE work. ----
            def head_pair(nt):
                hA, hB = 2 * nt, 2 * nt + 1
                oA0, oA1 = ps_o(), ps_o()
                oB0, oB1 = ps_o(), ps_o()
                rbs = {}

                def flush_pv(pa, jt, probs):
                    d0 = jt * 128
                    l0 = max(0, d0 - 512) if pa else d0
                    n1 = 512
                    for (h, o), p in zip(
                        ((hA, oA1 if pa else oA0), (hB, oB1 if pa else oB0)), probs
                    ):
                        nc.tensor.matmul(
                            o[:, l0:n1],
                            lhsT=VS[:, jt * VW + h * 128 : jt * VW + h * 128 + 128],
                            rhs=p[:, l0:n1],
                            start=(jt == 0),
                            stop=(jt == (NJ - 1 if pa else 3)),
                            skip_group_check=True,
                        )
                    if not pa and jt == 3:
                        rbs["A0"] = norm_prep(oA0)
                        rbs["B0"] = norm_prep(oB0)
                    if pa and jt == 5:
                        # broadcasts from pass 0 are long done; muls here keep
                        # the pair-end DVE queue short (and for the last pair
                        # unblock the out-projection's first i-blocks early)
                        norm_mul(oA0, rbs["A0"], hA, 0)
                        norm_mul(oB0, rbs["B0"], hB, 0)

                pending = None
                for pa, jts in ((0, range(4)), (1, range(NJ))):
                    g0 = 0 if pa == 0 else 512   # global col base of the pass
                    for jt in jts:
                        d0 = jt * 128
                        l0 = max(0, d0 - g0)     # first valid local col
                        sA, sB = ps_sc(), ps_sc()
                        pA = ppool.tile([128, 512], bf16, tag="probs", name="pA")
                        pB = ppool.tile([128, 512], bf16, tag="probs", name="pB")
                        for po, s in ((0, sA), (64, sB)):
                            nc.tensor.matmul(
                                s[:, l0:512],
                                lhsT=KS[po : po + 64,
                                        nt * T + d0 : nt * T + d0 + 128],
                                rhs=QS[po : po + 64,
                                       nt * T + g0 + l0 : nt * T + g0 + 512],
                                start=True,
                                stop=True,
                            )
                        for s, p in ((sA, pA), (sB, pB)):
                            nc.scalar.activation(
                                p[:, l0:512], s[:, l0:512], Exp, scale=SCALE
                            )
                        # causal triangle on the diag block (lands in this
                        # pass exactly when d0 is inside the pass's window).
                        # NOT gpsimd affine_select: mixing custom-op types on
                        # GpSimd forces MODIFY_POOL_CONFIG switches that
                        # stall partition_broadcast
                        if g0 <= d0 < g0 + 512:
                            for p in (pA, pB):
                                nc.vector.tensor_mul(
                                    p[:, l0 : l0 + 128], p[:, l0 : l0 + 128], TRI[:]
                                )
                        if pending is not None:
                            flush_pv(*pending)
                        pending = (pa, jt, (pA, pB))
                flush_pv(*pending)
                rbs["A1"] = norm_prep(oA1)
                rbs["B1"] = norm_prep(oB1)
                norm_mul(oA1, rbs["A1"], hA, 1)
                norm_mul(oB1, rbs["B1"], hB, 1)

            for nt in range(NT):
                qk_proj(nt)
                head_pair(nt)

            # ---- output projection: lhsT = O'[et, iblk] -> out[i, n]
            # directly. i-blocks 0..3 need only the ic0 normalizes (done at
            # pass-1 jt==5 of each pair), so they fill the PE while the last
            # pair's ic1 normalize chain drains; i-blocks 4..7 follow.
            for it in range(NJ):
                fin = fpool.tile([128, E], f32, tag="fin")
                for n0, nw in ((0, 512), (512, 256)):
                    f_ps = ps_sc()
                    for et in range(NT):
                        nc.tensor.matmul(
                            f_ps[:, :nw],
                            lhsT=OS[:, et * T + it * 128 : et * T + it * 128 + 128],
                            rhs=WO[:, et * E + n0 : et * E + n0 + nw],
                            start=(et == 0),
                            stop=(et == NT - 1),
                        )
                    nc.vector.tensor_add(
                        fin[:, n0 : n0 + nw], f_ps[:, :nw], FINB[:, n0 : n0 + nw]
                    )
                    (nc.sync if it % 2 == 0 else nc.scalar).dma_start(
                        out[it * 128 : (it + 1) * 128, n0 : n0 + nw],
                        fin[:, n0 : n0 + nw],
                    )

    nc.compile()
    return nc


def _get_nc():
    if "nc" not in _CACHE:
        _CACHE["nc"] = _build()
    return _CACHE["nc"]


def _make_in_maps(inputs):
    bf = ml_dtypes.bfloat16
    x = np.asarray(inputs["x"], np.float32)
    shared = {
        "wq": np.asarray(inputs["Wq"], np.float32).astype(bf),
        "wk": np.asarray(inputs["Wk"], np.float32).astype(bf),
        "wv": np.asarray(inputs["Wv"], np.float32).astype(bf),
        "wo": np.asarray(inputs["Wo"], np.float32).astype(bf),
        "bqt": np.ascontiguousarray(
            np.asarray(inputs["bq"], np.float32).reshape(NT, 128).T
        ),
        "bkt": np.ascontiguousarray(
            np.asarray(inputs["bk"], np.float32).reshape(NT, 128).T
        ),
        "bvr": np.asarray(inputs["bv"], np.float32).reshape(1, E).astype(bf),
        "bor": np.asarray(inputs["bo"], np.float32).reshape(1, E).astype(bf),
        "tri": np.triu(np.ones((128, 128), np.float32)).astype(bf),
        "ones12": np.ones((128, NJ * H), np.float32).astype(bf),
    }
    return [dict(shared, xt=x[b].T.astype(bf)) for b in range(B)]


def _run(inputs, trace=False):
    from concourse import bass_utils

    nc = _get_nc()
    res = bass_utils.run_bass_kernel_spmd(
        nc, _make_in_maps(inputs), core_ids=list(range(B)), trace=trace
    )
    out = np.stack([np.asarray(res.results[c]["out"]) for c in range(B)])
    return out, res


def kernel(**inputs) -> np.ndarray:
    out, _ = _run(inputs, trace=False)
    return out
